# revision 10
# baseline (speedup 1.0000x reference)
"""Trainium2 SPMD kernel for CGGRUNet (gnn_message_passing).

Sharding: edges sharded by dst-graph-group across 8 cores; node features
replicated via AllGather each conv step; GRU/Set2Set graph-sharded (8
graphs per core); per-edge matvec fused (We never materialized in DRAM).
"""
import sys
import numpy as np

sys.path.insert(0, "/opt/trn_rl_repo")

import concourse.bass as bass  # noqa: E402
import concourse.mybir as mybir  # noqa: E402
from concourse import bacc, tile, library_config  # noqa: E402
from concourse.tile_rust import add_dep_helper  # noqa: E402
from concourse.bass_utils import run_bass_kernel_spmd  # noqa: E402

F32 = mybir.dt.float32
I16 = mybir.dt.int16
I32 = mybir.dt.int32
Alu = mybir.AluOpType
Act = mybir.ActivationFunctionType
PSUM = bass.MemorySpace.PSUM

NC = 8          # cores
B = 64          # graphs
GPC = B // NC   # graphs per core
N = 10000       # nodes
E = 100000      # edges
DIM = 64
NF = 92
EF = 50
HH = 128
N_CONV = 2
S2S = 2
SLOTS = 1408    # node slots per core (11 * 128)
NTN = SLOTS // 128  # node tiles per core


def _pack_gather_idx(idx):
    """idx: int array len NT*128 -> [128, len/16] int16 in dma_gather layout.

    Measured device mapping: out[p, t] = G[p % 16, p // 16 + 8 * t] with the
    [16, len/16] block replicated 8x down partitions.
    """
    ni = len(idx)
    g = np.zeros((16, ni // 16), dtype=np.int16)
    j = np.arange(ni)
    r = (j % 128) % 16
    c = (j % 128) // 16 + 8 * (j // 128)
    g[r, c] = idx.astype(np.int16)
    return np.tile(g, (8, 1))


def _host_plan(edge_index, batch):
    src = np.asarray(edge_index[0]).astype(np.int64)
    dst = np.asarray(edge_index[1]).astype(np.int64)
    bt = np.asarray(batch).astype(np.int64)

    gcore = np.arange(B) // GPC          # graph -> core
    node_core = gcore[bt]                # node -> core (batch is sorted)
    core_start = np.searchsorted(node_core, np.arange(NC))
    core_end = np.searchsorted(node_core, np.arange(NC), side="right")
    ncnt = core_end - core_start
    assert ncnt.max() <= SLOTS, f"node shard {ncnt.max()} > SLOTS {SLOTS}"
    slot_of_node = np.arange(N) - core_start[node_core]
    gslot_of_node = node_core * SLOTS + slot_of_node

    deg = np.bincount(dst, minlength=N).astype(np.float64)
    inv_deg = np.where(deg > 0, 1.0 / np.maximum(deg, 1.0), 0.0).astype(np.float32)

    ecore = node_core[dst]
    per_core_eids = []
    NT = 1
    for c in range(NC):
        eids = np.nonzero(ecore == c)[0]
        order = np.argsort(slot_of_node[dst[eids]], kind="stable")
        eids = eids[order]
        per_core_eids.append(eids)
        NT = max(NT, (len(eids) + 127) // 128)
    EPC = NT * 128

    slot_mat = np.full((NC, EPC), -1, np.int64)     # dst slot per edge position
    for c in range(NC):
        e = per_core_eids[c]
        slot_mat[c, : len(e)] = slot_of_node[dst[e]]

    # per-tile global (across cores) min/max dst slot
    tmin = np.full(NT, SLOTS, np.int64)
    tmax = np.full(NT, -1, np.int64)
    for t in range(NT):
        sl = slot_mat[:, t * 128 : (t + 1) * 128]
        m = sl >= 0
        if m.any():
            tmin[t] = sl[m].min()
            tmax[t] = sl[m].max()

    # greedy runs of tiles sharing one anchor; window capped at 512
    WCAP = 512
    runs = []  # (t0, t1, anchor)
    t = 0
    while t < NT:
        a = int(128 * (min(tmin[t], SLOTS - 1) // 128))
        a = min(a, SLOTS - WCAP) if SLOTS >= WCAP else 0
        t1 = t
        while t1 < NT and (tmax[t1] < a + WCAP) and (tmin[t1] >= a or tmax[t1] < 0):
            t1 += 1
        assert t1 > t, f"tile {t}: slots [{tmin[t]},{tmax[t]}] don't fit anchor {a}"
        runs.append((t, t1, a))
        t = t1
    wneed = 1
    for (t0, t1, a) in runs:
        for t in range(t0, t1):
            if tmax[t] >= 0:
                wneed = max(wneed, int(tmax[t] - a + 1))
    W = int(128 * ((wneed + 127) // 128))
    assert W <= WCAP

    anchor_of_tile = np.zeros(NT, np.int64)
    for (t0, t1, a) in runs:
        anchor_of_tile[t0:t1] = a

    plan = dict(NT=NT, EPC=EPC, W=W, runs=runs,
                core_start=core_start, ncnt=ncnt,
                slot_of_node=slot_of_node, gslot_of_node=gslot_of_node,
                inv_deg=inv_deg, per_core_eids=per_core_eids,
                slot_mat=slot_mat, anchor_of_tile=anchor_of_tile,
                src=src, dst=dst, bt=bt)
    return plan


def _build_in_maps(plan, inputs):
    NT, EPC, W = plan["NT"], plan["EPC"], plan["W"]
    src, dst, bt = plan["src"], plan["dst"], plan["bt"]
    x = np.asarray(inputs["x"], np.float32)
    ea = np.asarray(inputs["edge_attr"], np.float32)

    f32 = lambda a: np.ascontiguousarray(np.asarray(a, np.float32))
    col = lambda a: f32(a).reshape(-1, 1)

    # shared (per-core identical) tensors
    shared = {
        "lin0_wT": f32(inputs["lin0_w"]).T.copy(),           # [NF, DIM]
        "lin0_b": col(inputs["lin0_b"]),                     # [DIM,1]
        "nn1_wT": f32(inputs["nn1_w"]).T.copy(),             # [EF, HH]
        "nn1_b": col(inputs["nn1_b"]),                       # [HH,1]
        "nn2_wT": f32(inputs["nn2_w"]).T.copy(),             # [HH, DIM*DIM]
        "rootm": f32(inputs["root"]),                        # [DIM, DIM] lhsT k=i m=o
        "conv_b": col(inputs["conv_bias"]),                  # [DIM,1]
        "gw_ih_rT": f32(inputs["gru_w_ih"][0:DIM]).T.copy(),       # [DIM, DIM]
        "gw_ih_zT": f32(inputs["gru_w_ih"][DIM:2 * DIM]).T.copy(),
        "gw_ih_nT": f32(inputs["gru_w_ih"][2 * DIM:]).T.copy(),
        "gw_hh_rT": f32(inputs["gru_w_hh"][0:DIM]).T.copy(),
        "gw_hh_zT": f32(inputs["gru_w_hh"][DIM:2 * DIM]).T.copy(),
        "gw_hh_nT": f32(inputs["gru_w_hh"][2 * DIM:]).T.copy(),
        "g_b_r": col(np.asarray(inputs["gru_b_ih"], np.float64)[0:DIM]
                     + np.asarray(inputs["gru_b_hh"], np.float64)[0:DIM]),
        "g_b_z": col(np.asarray(inputs["gru_b_ih"], np.float64)[DIM:2 * DIM]
                     + np.asarray(inputs["gru_b_hh"], np.float64)[DIM:2 * DIM]),
        "g_b_ihn": col(np.asarray(inputs["gru_b_ih"])[2 * DIM:]),
        "g_b_hhn": col(np.asarray(inputs["gru_b_hh"])[2 * DIM:]),
        "lw_ihT_q": f32(inputs["lstm_w_ih"][:, 0:DIM]).T.copy(),   # [DIM, 4DIM]
        "lw_ihT_r": f32(inputs["lstm_w_ih"][:, DIM:]).T.copy(),    # [DIM, 4DIM]
        "lw_hhT": f32(inputs["lstm_w_hh"]).T.copy(),               # [DIM, 4DIM]
        "l_b": (np.asarray(inputs["lstm_b_ih"], np.float64)
                + np.asarray(inputs["lstm_b_hh"], np.float64)
                ).astype(np.float32).reshape(4, DIM).T.copy(),     # [DIM,4]
        "lin1_wT_q": f32(inputs["lin1_w"][:, 0:DIM]).T.copy(),     # [DIM, DIM]
        "lin1_wT_r": f32(inputs["lin1_w"][:, DIM:]).T.copy(),
        "lin1_b": col(inputs["lin1_b"]),
        "lin2_wT": f32(inputs["lin2_w"]).T.copy(),           # [DIM, 1]
        "lin2_b": col(inputs["lin2_b"]),
        "ident": np.eye(128, dtype=np.float32),
        "nn2_B": f32(inputs["nn2_b"]).reshape(DIM, DIM),     # [i, o] lhsT k=i m=o
    }

    in_maps = []
    for c in range(NC):
        n0 = plan["core_start"][c]
        nc_cnt = plan["ncnt"][c]
        eids = plan["per_core_eids"][c]
        ne = len(eids)

        xT = np.zeros((NF, SLOTS), np.float32)
        xT[:, :nc_cnt] = x[n0 : n0 + nc_cnt].T

        eaT = np.zeros((EF, EPC), np.float32)
        eaT[:, :ne] = ea[eids].T

        dstrel = np.full(EPC, -1.0, np.float32)
        dstrel[:ne] = (plan["slot_mat"][c, :ne]
                       - plan["anchor_of_tile"][np.arange(ne) // 128]).astype(np.float32)
        assert dstrel[:ne].min() >= 0 and dstrel[:ne].max() < W

        invdeg_e = np.zeros(EPC, np.float32)
        invdeg_e[:ne] = plan["inv_deg"][dst[eids]]

        gsl = np.zeros(EPC, np.int64)
        gsl[:ne] = plan["gslot_of_node"][src[eids]]

        gmem = bt[n0 : n0 + nc_cnt] - c * GPC     # local graph of each real slot
        G = np.zeros((GPC, SLOTS), np.float32)
        G[gmem, np.arange(nc_cnt)] = 1.0
        GT = G.T.reshape(NTN, 128, GPC).transpose(1, 0, 2).copy()  # [128, NTN, GPC]

        m = dict(shared)
        m.update({
            "xT": xT,
            "eaT": eaT,
            "dstrel": dstrel.reshape(NT, 128).T.copy(),      # [128, NT]
            "invdeg_e": invdeg_e.reshape(NT, 128).T.copy(),  # [128, NT]
            "gidx": _pack_gather_idx(gsl),                   # [128, EPC//16]
            "G": G,
            "GT": GT,
        })
        in_maps.append(m)
    return in_maps


def _build_kernel(NT, W, with_nn2b):
    EPC = NT * 128
    nc = bacc.Bacc("TRN2", target_bir_lowering=False, debug=False, num_devices=NC)

    # ---- dram I/O ----
    d_in = {}
    def din(name, shape, dtype=F32):
        d_in[name] = nc.dram_tensor(name, list(shape), dtype, kind="ExternalInput")
        return d_in[name]

    din("xT", (NF, SLOTS))
    ea_dram = nc.dram_tensor("eaT", [EF, EPC], F32, kind="ExternalInput")
    din("lin0_wT", (NF, DIM)); din("lin0_b", (DIM, 1))
    din("nn1_wT", (EF, HH)); din("nn1_b", (HH, 1))
    din("nn2_wT", (HH, DIM * DIM))
    din("rootm", (DIM, DIM)); din("conv_b", (DIM, 1))
    for nm in ("gw_ih_rT", "gw_ih_zT", "gw_ih_nT", "gw_hh_rT", "gw_hh_zT", "gw_hh_nT"):
        din(nm, (DIM, DIM))
    din("g_b_r", (DIM, 1)); din("g_b_z", (DIM, 1))
    din("g_b_ihn", (DIM, 1)); din("g_b_hhn", (DIM, 1))
    din("lw_ihT_q", (DIM, 4 * DIM)); din("lw_ihT_r", (DIM, 4 * DIM))
    din("lw_hhT", (DIM, 4 * DIM)); din("l_b", (DIM, 4))
    din("lin1_wT_q", (DIM, DIM)); din("lin1_wT_r", (DIM, DIM)); din("lin1_b", (DIM, 1))
    din("lin2_wT", (DIM, 1)); din("lin2_b", (1, 1))
    din("ident", (128, 128))
    din("dstrel", (128, NT)); din("invdeg_e", (128, NT))
    din("gidx", (128, EPC // 16), I16)
    din("G", (GPC, SLOTS)); din("GT", (128, NTN, GPC))
    if with_nn2b:
        din("nn2_B", (DIM, DIM))

    y_out = nc.dram_tensor("y", [1, GPC], F32, kind="ExternalOutput")

    h_shard = [nc.dram_tensor(f"h_shard{s}", [SLOTS, DIM], F32) for s in range(N_CONV)]
    h_full = [nc.dram_tensor(f"h_full{s}", [NC * SLOTS, DIM], F32, addr_space="Shared")
              for s in range(N_CONV)]

    nc.gpsimd.load_library(library_config.mlp)

    from contextlib import ExitStack
    with tile.TileContext(nc) as tc, ExitStack() as _ex:
        konst = _ex.enter_context(tc.tile_pool(name="konst", bufs=1))
        pers = _ex.enter_context(tc.tile_pool(name="pers", bufs=1))

        # ---- load constants into SBUF ----
        kt = {}
        for name, t in d_in.items():
            sh = list(t.shape)
            kt[name] = konst.tile(sh, t.dtype, tag=f"k_{name}", name=f"k_{name}")
            nc.sync.dma_start(kt[name][:], t[:])

        # iota row 0..W-1 (f32) for S-tile building
        iota_i = konst.tile([128, W], I32, tag="iota_i")
        nc.gpsimd.iota(iota_i[:], pattern=[[1, W]], base=0, channel_multiplier=0)
        iota_f = konst.tile([128, W], F32, tag="iota_f")
        nc.vector.tensor_copy(iota_f[:], iota_i[:])

        # ---- persistent state ----
        he_T = pers.tile([HH, EPC], F32, tag="he_T")
        h_T = pers.tile([DIM, SLOTS], F32, tag="h_T")
        h_rows = pers.tile([128, NTN, DIM], F32, tag="h_rows")
        agg_T = pers.tile([DIM, SLOTS], F32, tag="agg_T")
        gath = pers.tile([128, NT, DIM], F32, tag="gath")

        # ---- he_T = relu(nn1 @ eaT + b) ----
        with (
            tc.tile_pool(name="ps_he", bufs=3, space=PSUM) as ps,
            tc.tile_pool(name="ea_wk", bufs=3) as eawk,
        ):
            for c0 in range(0, EPC, 512):
                cw = min(512, EPC - c0)
                eat = eawk.tile([EF, 512], F32, tag="eat")
                nc.sync.dma_start(eat[:, :cw], ea_dram[:, c0:c0 + cw])
                pt = ps.tile([HH, 512], F32, tag="he")
                nc.tensor.matmul(pt[:, :cw], kt["nn1_wT"][:], eat[:, :cw],
                                 start=True, stop=True)
                nc.scalar.activation(he_T[:, c0:c0 + cw], pt[:, :cw], Act.Relu,
                                     bias=kt["nn1_b"][:])

        # ---- h0 = relu(lin0 @ xT + b) ----
        with tc.tile_pool(name="ps_l0", bufs=3, space=PSUM) as ps:
            for c0 in range(0, SLOTS, 512):
                cw = min(512, SLOTS - c0)
                pt = ps.tile([DIM, 512], F32, tag="l0")
                nc.tensor.matmul(pt[:, :cw], kt["lin0_wT"][:], kt["xT"][:, c0:c0 + cw],
                                 start=True, stop=True)
                nc.scalar.activation(h_T[:, c0:c0 + cw], pt[:, :cw], Act.Relu,
                                     bias=kt["lin0_b"][:])

        def publish_h(step):
            """h_T -> h_rows -> dram shard -> AllGather h_full[step]."""
            with tc.tile_pool(name=f"ps_tr{step}", bufs=3, space=PSUM) as ps:
                for tn in range(NTN):
                    pt = ps.tile([128, DIM], F32, tag="tr")
                    nc.tensor.transpose(pt[:], h_T[:, tn * 128:(tn + 1) * 128],
                                        kt["ident"][:DIM, :DIM])
                    nc.vector.tensor_copy(h_rows[:, tn, :], pt[:])
            dma = nc.sync.dma_start(
                h_shard[step][:].rearrange("(tn p) d -> p tn d", p=128), h_rows[:])
            cc = nc.gpsimd.collective_compute(
                "AllGather", Alu.bypass,
                ins=[h_shard[step][:]], outs=[h_full[step][:]],
                replica_groups=[list(range(NC))],
            )
            add_dep_helper(cc.ins, dma.ins, sync=True, reason="AG waits shard dma")
            return cc

        # chunking of the (i,o) = DIM*DIM axis for the fused matvec
        CH = [(0, 1536), (1536, 3072), (3072, 4096)]

        for step in range(N_CONV):
            cc = publish_h(step)
            g = nc.gpsimd.dma_gather(
                out_ap=gath[:], in_ap=h_full[step][:], idxs_ap=kt["gidx"][:],
                num_idxs=EPC, num_idxs_reg=EPC, elem_size=DIM,
                single_packet=False,
            )
            add_dep_helper(g.ins, cc.ins, sync=True, reason="gather waits AG")

            nc.gpsimd.memset(agg_T[:], 0.0)

            with (
                tc.tile_pool(name=f"ps_we{step}", bufs=2, space=PSUM) as pwe,
                tc.tile_pool(name=f"ps_s{step}", bufs=2, space=PSUM) as pss,
                tc.tile_pool(name=f"wk{step}", bufs=3) as wk,
            ):
                for (t0, t1, anchor) in _RUNS:
                    ps_s = pss.tile([DIM, W], F32, tag="ps_s")
                    for t in range(t0, t1):
                        esl = slice(t * 128, (t + 1) * 128)
                        # scaled source rows: h[src] * inv_deg[dst]
                        hs = wk.tile([128, DIM], F32, tag="hs")
                        nc.vector.tensor_scalar(
                            hs[:], gath[:, t, :], kt["invdeg_e"][:, t:t + 1], None,
                            op0=Alu.mult)
                        # S tile from dstrel + iota
                        s_t = wk.tile([128, W], F32, tag="s_t")
                        nc.gpsimd.tensor_scalar(
                            s_t[:], iota_f[:], kt["dstrel"][:, t:t + 1], None,
                            op0=Alu.is_equal)
                        # fused We-matmul + matvec
                        parts = []
                        for (c0, c1) in CH:
                            cw = c1 - c0
                            ni = cw // DIM
                            pwe_t = pwe.tile([128, 1536], F32, tag="we")
                            for m0 in range(0, cw, 512):
                                nc.tensor.matmul(
                                    pwe_t[:, m0:m0 + 512], he_T[:, esl],
                                    kt["nn2_wT"][:, c0 + m0:c0 + m0 + 512],
                                    start=True, stop=True)
                            pchunk = wk.tile([128, 1536], F32, tag="pchunk")
                            hs_b = (hs[:, c0 // DIM:c1 // DIM]
                                    .unsqueeze(2).broadcast_to([128, ni, DIM]))
                            nc.vector.tensor_tensor(
                                pchunk[:, :cw].rearrange("p (i o) -> p i o", i=ni),
                                pwe_t[:, :cw].rearrange("p (i o) -> p i o", i=ni),
                                hs_b, op=Alu.mult)
                            part = wk.tile([128, DIM], F32, tag="part")
                            nc.vector.tensor_reduce(
                                part[:],
                                pchunk[:, :cw].rearrange("p (i o) -> p o i", i=ni),
                                axis=mybir.AxisListType.X, op=Alu.add)
                            parts.append(part)
                        msg = wk.tile([128, DIM], F32, tag="msg")
                        nc.vector.tensor_tensor(msg[:], parts[0][:], parts[1][:],
                                                op=Alu.add)
                        nc.vector.tensor_tensor(msg[:], msg[:], parts[2][:],
                                                op=Alu.add)
                        # scatter: agg_T[:, a:a+W] (psum) += msg.T @ S
                        nc.tensor.matmul(ps_s[:], msg[:], s_t[:],
                                         start=(t == t0), stop=(t == t1 - 1))
                    wa = min(anchor, SLOTS - W)
                    nc.vector.tensor_tensor(agg_T[:, wa:wa + W], agg_T[:, wa:wa + W],
                                            ps_s[:], op=Alu.add)

            # ---- m = relu(agg*invdeg + h@root + bias);  GRU ----
            with (
                tc.tile_pool(name=f"ps_m{step}", bufs=1, space=PSUM) as psm,
                tc.tile_pool(name=f"wkm{step}", bufs=1) as wkm,
            ):
                rootp = psm.tile([DIM, SLOTS], F32, tag="psA")
                for c0 in range(0, SLOTS, 512):
                    cw = min(512, SLOTS - c0)
                    nc.tensor.matmul(rootp[:, c0:c0 + cw], kt["rootm"][:],
                                     h_T[:, c0:c0 + cw], start=True,
                                     stop=not with_nn2b)
                if with_nn2b:
                    # ssum path: agg_bias = (scatter-sum of scaled h_src) @ B
                    # (skipped when nn2_b == 0; see _build_in_maps)
                    pass
                m_T = wkm.tile([DIM, SLOTS], F32, tag="m_T")
                nc.vector.tensor_tensor(m_T[:], agg_T[:], rootp[:], op=Alu.add)
                nc.scalar.activation(m_T[:], m_T[:], Act.Relu, bias=kt["conv_b"][:])

                # GRU gates (r, z, n) all on partitions 0:64
                def gate_mm(ih, hh):
                    p = psm.tile([DIM, SLOTS], F32, tag="psA", name="gatep")
                    for c0 in range(0, SLOTS, 512):
                        cw = min(512, SLOTS - c0)
                        nc.tensor.matmul(p[:, c0:c0 + cw], kt[ih][:],
                                         m_T[:, c0:c0 + cw], start=True, stop=False)
                        nc.tensor.matmul(p[:, c0:c0 + cw], kt[hh][:],
                                         h_T[:, c0:c0 + cw], start=False, stop=True)
                    return p

                rp = gate_mm("gw_ih_rT", "gw_hh_rT")
                rg = wkm.tile([DIM, SLOTS], F32, tag="rg")
                nc.scalar.activation(rg[:], rp[:], Act.Sigmoid, bias=kt["g_b_r"][:])
                zp = gate_mm("gw_ih_zT", "gw_hh_zT")
                zg = wkm.tile([DIM, SLOTS], F32, tag="zg")
                nc.scalar.activation(zg[:], zp[:], Act.Sigmoid, bias=kt["g_b_z"][:])

                hnp = psm.tile([DIM, SLOTS], F32, tag="psB")
                for c0 in range(0, SLOTS, 512):
                    cw = min(512, SLOTS - c0)
                    nc.tensor.matmul(hnp[:, c0:c0 + cw], kt["gw_hh_nT"][:],
                                     h_T[:, c0:c0 + cw], start=True, stop=True)
                hn = wkm.tile([DIM, SLOTS], F32, tag="hn")
                nc.scalar.activation(hn[:], hnp[:], Act.Identity, bias=kt["g_b_hhn"][:])
                xnp = psm.tile([DIM, SLOTS], F32, tag="psA")
                for c0 in range(0, SLOTS, 512):
                    cw = min(512, SLOTS - c0)
                    nc.tensor.matmul(xnp[:, c0:c0 + cw], kt["gw_ih_nT"][:],
                                     m_T[:, c0:c0 + cw], start=True, stop=True)
                ng = wkm.tile([DIM, SLOTS], F32, tag="ng")
                nc.vector.tensor_tensor(ng[:], rg[:], hn[:], op=Alu.mult)
                nc.vector.tensor_tensor(ng[:], ng[:], xnp[:], op=Alu.add)
                nc.scalar.activation(ng[:], ng[:], Act.Tanh, bias=kt["g_b_ihn"][:])
                # h' = ng + z*(h - ng)
                tmp = wkm.tile([DIM, SLOTS], F32, tag="tmp")
                nc.vector.tensor_tensor(tmp[:], h_T[:], ng[:], op=Alu.subtract)
                nc.vector.tensor_tensor(tmp[:], zg[:], tmp[:], op=Alu.mult)
                nc.vector.tensor_tensor(h_T[:], ng[:], tmp[:], op=Alu.add)

        # ---- final h_rows refresh for set2set ----
        with tc.tile_pool(name="ps_trf", bufs=3, space=PSUM) as ps:
            for tn in range(NTN):
                pt = ps.tile([128, DIM], F32, tag="trf")
                nc.tensor.transpose(pt[:], h_T[:, tn * 128:(tn + 1) * 128],
                                    kt["ident"][:DIM, :DIM])
                nc.vector.tensor_copy(h_rows[:, tn, :], pt[:])

        # ---- Set2Set over GPC local graphs ----
        with (
            tc.tile_pool(name="s2s", bufs=1) as sp,
            tc.tile_pool(name="ps_s2s", bufs=1, space=PSUM) as ps,
            tc.tile_pool(name="ps_qb", bufs=1, space=PSUM) as psq,
        ):
            qstarT_q = sp.tile([DIM, GPC], F32, tag="qstarT_q")
            qstarT_r = sp.tile([DIM, GPC], F32, tag="qstarT_r")
            hsT = sp.tile([DIM, GPC], F32, tag="hsT")
            csT = sp.tile([DIM, GPC], F32, tag="csT")
            nc.gpsimd.memset(qstarT_q[:], 0.0)
            nc.gpsimd.memset(qstarT_r[:], 0.0)
            nc.gpsimd.memset(hsT[:], 0.0)
            nc.gpsimd.memset(csT[:], 0.0)

            for s in range(S2S):
                # LSTM cell, one [64]-partition psum per gate (order i,f,g,o)
                def lstm_gate(gi):
                    c0 = gi * DIM
                    p = ps.tile([DIM, GPC], F32, tag=f"lg{gi % 3}", name="lgp")
                    nc.tensor.matmul(p[:], kt["lw_ihT_q"][:, c0:c0 + DIM],
                                     qstarT_q[:], start=True, stop=False)
                    nc.tensor.matmul(p[:], kt["lw_ihT_r"][:, c0:c0 + DIM],
                                     qstarT_r[:], start=False, stop=False)
                    nc.tensor.matmul(p[:], kt["lw_hhT"][:, c0:c0 + DIM],
                                     hsT[:], start=False, stop=True)
                    return p

                ig = sp.tile([DIM, GPC], F32, tag="ig")
                nc.scalar.activation(ig[:], lstm_gate(0), Act.Sigmoid,
                                     bias=kt["l_b"][:, 0:1])
                fg = sp.tile([DIM, GPC], F32, tag="fg")
                nc.scalar.activation(fg[:], lstm_gate(1), Act.Sigmoid,
                                     bias=kt["l_b"][:, 1:2])
                tg = sp.tile([DIM, GPC], F32, tag="tg")
                nc.scalar.activation(tg[:], lstm_gate(2), Act.Tanh,
                                     bias=kt["l_b"][:, 2:3])
                og = sp.tile([DIM, GPC], F32, tag="og")
                nc.scalar.activation(og[:], lstm_gate(3), Act.Sigmoid,
                                     bias=kt["l_b"][:, 3:4])
                nc.vector.tensor_tensor(csT[:], fg[:], csT[:], op=Alu.mult)
                tmp2 = sp.tile([DIM, GPC], F32, tag="tmp2")
                nc.vector.tensor_tensor(tmp2[:], ig[:], tg[:], op=Alu.mult)
                nc.vector.tensor_tensor(csT[:], csT[:], tmp2[:], op=Alu.add)
                tc_ = sp.tile([DIM, GPC], F32, tag="tc_")
                nc.scalar.activation(tc_[:], csT[:], Act.Tanh)
                nc.vector.tensor_tensor(hsT[:], og[:], tc_[:], op=Alu.mult)

                # attention: e[s] = h[s] . q[graph(s)]
                hs_rows_p = ps.tile([GPC, DIM], F32, tag="hsrow")
                nc.tensor.transpose(hs_rows_p[:], hsT[:], kt["ident"][:DIM, :DIM])
                hs_rows = sp.tile([GPC, DIM], F32, tag="hsrows")
                nc.vector.tensor_copy(hs_rows[:], hs_rows_p[:])

                ee_col = sp.tile([128, NTN], F32, tag="ee_col")
                for tn in range(NTN):
                    qb = psq.tile([128, DIM], F32, tag="qb")
                    nc.tensor.matmul(qb[:], kt["G"][:, tn * 128:(tn + 1) * 128],
                                     hs_rows[:], start=True, stop=True)
                    pe_t = sp.tile([128, DIM], F32, tag="pe_t")
                    nc.vector.tensor_tensor(pe_t[:], h_rows[:, tn, :], qb[:],
                                            op=Alu.mult)
                    nc.vector.tensor_reduce(ee_col[:, tn:tn + 1], pe_t[:],
                                            axis=mybir.AxisListType.X, op=Alu.add)
                nc.scalar.activation(ee_col[:], ee_col[:], Act.Exp)

                denp = ps.tile([GPC, 1], F32, tag="denp")
                for tn in range(NTN):
                    nc.tensor.matmul(denp[:], kt["GT"][:, tn, :],
                                     ee_col[:, tn:tn + 1],
                                     start=(tn == 0), stop=(tn == NTN - 1))
                invden = sp.tile([GPC, 1], F32, tag="invden")
                nc.vector.reciprocal(invden[:], denp[:])
                # note: reference divides by max(denom, 1e-16); denom >= per-graph
                # node count * min exp > 0 here, so plain reciprocal matches.

                idb = ps.tile([128, NTN], F32, tag="idb")
                for tn in range(NTN):
                    nc.tensor.matmul(idb[:, tn:tn + 1],
                                     kt["G"][:, tn * 128:(tn + 1) * 128], invden[:],
                                     start=True, stop=True)
                a_col = sp.tile([128, NTN], F32, tag="a_col")
                nc.vector.tensor_tensor(a_col[:], ee_col[:], idb[:], op=Alu.mult)

                rvp = ps.tile([DIM, GPC], F32, tag="rvp")
                for tn in range(NTN):
                    agt = sp.tile([128, GPC], F32, tag="agt")
                    nc.vector.tensor_scalar(agt[:], kt["GT"][:, tn, :],
                                            a_col[:, tn:tn + 1], None, op0=Alu.mult)
                    nc.tensor.matmul(rvp[:], h_rows[:, tn, :], agt[:],
                                     start=(tn == 0), stop=(tn == NTN - 1))
                nc.vector.tensor_copy(qstarT_q[:], hsT[:])
                nc.vector.tensor_copy(qstarT_r[:], rvp[:])

            # ---- output head ----
            y1p = ps.tile([DIM, GPC], F32, tag="lg0")
            nc.tensor.matmul(y1p[:], kt["lin1_wT_q"][:], qstarT_q[:],
                             start=True, stop=False)
            nc.tensor.matmul(y1p[:], kt["lin1_wT_r"][:], qstarT_r[:],
                             start=False, stop=True)
            y1 = sp.tile([DIM, GPC], F32, tag="y1")
            nc.scalar.activation(y1[:], y1p[:], Act.Relu, bias=kt["lin1_b"][:])
            y2p = ps.tile([1, GPC], F32, tag="lg1")
            nc.tensor.matmul(y2p[:], kt["lin2_wT"][:], y1[:], start=True, stop=True)
            y2 = sp.tile([1, GPC], F32, tag="y2")
            nc.scalar.activation(y2[:], y2p[:], Act.Identity, bias=kt["lin2_b"][:])
            nc.sync.dma_start(y_out[:], y2[:])

    nc.finalize()
    return nc


_RUNS = None  # set per build (closure hack kept simple)


def kernel(**inputs):
    global _RUNS
    plan = _host_plan(inputs["edge_index"], inputs["batch"])
    in_maps = _build_in_maps(plan, inputs)
    with_nn2b = bool(np.any(np.asarray(inputs["nn2_b"]) != 0))
    assert not with_nn2b, "nn2_b != 0 path not implemented (spec fills zeros)"
    _RUNS = plan["runs"]
    nc = _build_kernel(plan["NT"], plan["W"], with_nn2b)
    res = run_bass_kernel_spmd(nc, in_maps, list(range(NC)))
    y = np.zeros(B, np.float32)
    for c in range(NC):
        y[c * GPC:(c + 1) * GPC] = res.results[c]["y"][0]
    return y
